# revision 17
# baseline (speedup 1.0000x reference)
"""AffinityPropagate Trainium2 kernel.

Problem: 24 iterations of a per-pixel-weighted 3x3 stencil (zero-padded)
on a [B=8, C=1, H=256, W=1216] image, weights = abs-normalized affinity
[B, 9, H, W].  Data-parallel over batch: one image per NeuronCore.

Per-core algorithm
------------------
Row i of the image maps to (partition p = i//2, slot c = i%2), so the
256 rows live on 128 partitions x 2 free-dim blocks.  With that mapping
a +-1 row shift never crosses a partition-chunk seam: it is a partition
shift by one (handled exactly by a 128x128 shifted-identity matmul,
zero-padding falls out of the missing matrix row) and/or a slot swap
(pure free-dim addressing).

Weights are normalized once, then pre-shifted so the per-iteration inner
loop is only:
  DVE : z[n]  = wsh[n] * f          (9 taps in one tensor_tensor via a
                                     stride-0 broadcast of f; fp16 -> 2x mode)
  PE  : out[c] += S_{n,c} @ z[n][shifted columns]   (PSUM fp32 accumulate)
  ACT : f' = cast(out)              (PSUM -> SBUF fp16 copy)
All column (W) shifts are plain free-dim address offsets into z's
guard-padded blocks; all row (H) shifts are the stationary matrices.

Preamble: inputs ship as fp16 in the SBUF-resident layout (half the HBM
traffic, fully contiguous DMA lines), |a| is a 4x-mode DVE
tensor_scalar(bitwise_and 0x7fff), the 9-map normalizer sum runs on the
otherwise-idle PE as accumulating identity matmuls, and the
normalize/pre-shift build is split band 0 vs rest with iteration 1's
first TT emitted in between, so the loop is running while the last two
thirds of wsh are still being built.
"""

import os
import sys

import numpy as np

for _p in ("/opt/trn_rl_repo", "/opt/pypackages"):
    if os.path.isdir(_p) and _p not in sys.path:
        sys.path.insert(0, _p)

B, K2, H, W = 8, 9, 256, 1216
P = 128          # partitions; row i -> (p=i//2, c=i%2)
NS = 2           # row slots per partition
GU = 2           # guard columns on each side of a z block (4B aligned)
WB = W + 2 * GU  # z/wsh block width
TILES = [(0, 406), (406, 406), (812, 404)]  # PSUM bank tiling (even sizes)
# TT tiling for steady-state iterations: fewer DVE ops (the per-op fixed
# cost is ~150 cycles), still enough split that PE/ACT finish tile 0 and
# hand the next iteration its feature tile before the second TT ends.
TT_TILES = [(0, 812), (812, 404)]

# Per-bank matmul plan: out slot c' accumulates, for each tap n,
# z[n][src slot] routed through stationary matrix:
#   s = di-1 = -1: c'=0 <- (S_dn, slot 1),  c'=1 <- (I, slot 0)
#   s = 0        : c' <- (I, slot c')
#   s = +1       : c'=0 <- (I, slot 1),  c'=1 <- (S_up, slot 0)
# Stationary index: 0 = S_dn (k == m-1), 1 = I, 2 = S_up (k == m+1).
# Ordered so consecutive matmuls mostly share the stationary operand.
PLAN = {
    0: [(0, 1, 0), (1, 1, 0), (2, 1, 0),
        (3, 0, 1), (4, 0, 1), (5, 0, 1),
        (6, 1, 1), (7, 1, 1), (8, 1, 1)],
    1: [(0, 0, 1), (1, 0, 1), (2, 0, 1),
        (3, 1, 1), (4, 1, 1), (5, 1, 1),
        (6, 0, 2), (7, 0, 2), (8, 0, 2)],
}

# Taps with a row shift (di != 1).
SHIFT_TAPS = (0, 1, 2, 6, 7, 8)

_CACHE = {}


def _shift_mats() -> np.ndarray:
    """[3, 128, 128] fp16: S_dn (k==m-1), I, S_up (k==m+1) as lhsT[k, m]."""
    s = np.zeros((3, P, P), dtype=np.float16)
    k = np.arange(P - 1)
    s[0][k, k + 1] = 1.0      # out[m] = mov[m-1]
    s[1][np.arange(P), np.arange(P)] = 1.0
    s[2][k + 1, k] = 1.0      # out[m] = mov[m+1]
    return s


def build_program(times: int):
    import concourse.bacc as bacc
    import concourse.tile as tile
    from concourse import mybir
    from contextlib import ExitStack

    dt = mybir.dt
    nc = bacc.Bacc(trn_type="TRN2", target_bir_lowering=False, debug=False,
                   num_devices=B)

    # aff ships already transposed to the SBUF layout [p, n, c, j]: each
    # partition's slice is one contiguous DRAM run -> max DMA efficiency.
    aff = nc.dram_tensor("aff", [P, K2, NS, W], dt.float16,
                         kind="ExternalInput")
    feat = nc.dram_tensor("feat", [H, W], dt.float16, kind="ExternalInput")
    smat = nc.dram_tensor("smat", [3, P, P], dt.float16, kind="ExternalInput")
    out = nc.dram_tensor("out", [H, W], dt.float32, kind="ExternalOutput")

    feat_r = feat.ap().rearrange("(p c) j -> p c j", c=NS)
    out_r = out.ap().rearrange("(p c) j -> p c j", c=NS)

    with tile.TileContext(nc) as tc, ExitStack() as ctx:
        persist = ctx.enter_context(tc.tile_pool(name="persist", bufs=1))
        fpool = ctx.enter_context(tc.tile_pool(name="fpool", bufs=2))

        smat_t = persist.tile([P, 3, P], dt.float16)
        wsh = persist.tile([P, K2, NS, WB], dt.float16)
        # z buffer 0 lives outside the preamble pools so iteration 1's TT
        # does not wait for the preamble-pool release.  z guard columns are
        # never initialized: the edge matmuls clamp their column ranges
        # instead (the clamped-away taps are exactly the zero-padding).
        z0 = persist.tile([P, K2, NS, WB], dt.float16, name="z0")

        f0 = fpool.tile([P, NS, W], dt.float16, tag="f")

        # ---- loop emission helpers (iteration 1's TTs are interleaved
        # with the preamble build below) ----
        dj01 = {c: [e for e in PLAN[c] if e[0] % 3 <= 1] for c in (0, 1)}
        dj2 = {c: [e for e in PLAN[c] if e[0] % 3 == 2] for c in (0, 1)}
        # Tile 0 starts its accumulation group with a dj=1 tap (full column
        # range) so the edge-clamped dj=0 taps never own the start flag;
        # order keeps stationary runs contiguous (no extra LDWEIGHTS).
        tap_order = {0: [4, 3, 6, 7, 0, 1], 1: [4, 3, 0, 1, 6, 7]}
        dj01_t0 = {c: sorted(dj01[c], key=lambda e: tap_order[c].index(e[0]))
                   for c in (0, 1)}

        def emit_tt(z, f_cur, t0, L):
            fb = f_cur[:, :, t0:t0 + L].unsqueeze(1).broadcast_to(
                [P, K2, NS, L])
            nc.vector.tensor_tensor(
                out=z[:, :, :, GU + t0:GU + t0 + L],
                in0=wsh[:, :, :, GU + t0:GU + t0 + L],
                in1=fb, op=mybir.AluOpType.mult)

        with tc.tile_pool(name="pre", bufs=1) as prep, \
                tc.tile_pool(name="wstage", bufs=1) as wstagep, \
                tc.tile_pool(name="accps", bufs=6, space="PSUM") as accp, \
                tc.tile_pool(name="prepsum", bufs=2, space="PSUM") as prepsum:
            absa = prep.tile([P, K2, NS, W], dt.float16)
            rcp32 = prep.tile([P, NS, W], dt.float32)
            rcp = prep.tile([P, NS, W], dt.float16)

            # wsh cells the build never writes (guards + the one edge
            # column of each shifted tap) must be zero: they feed the loop
            # TT and the shift matmuls would accumulate garbage otherwise.
            nc.vector.memset(wsh[:, :, :, 0:GU + 1], 0.0)
            nc.vector.memset(wsh[:, :, :, GU + W - 1:WB], 0.0)

            # Affinity lands straight in absa: 4 DMAs so |a| can start
            # after the first two maps arrive.  Feature and the shift
            # matrices ride separate queues.
            batches = [(0, 2), (2, 2), (4, 2), (6, 3)]
            for n0, cnt in batches:
                nc.sync.dma_start(out=absa[:, n0:n0 + cnt],
                                  in_=aff.ap()[:, n0:n0 + cnt])
            nc.sync.dma_start(out=f0, in_=feat_r)
            nc.gpsimd.dma_start(out=smat_t, in_=smat.ap().transpose([1, 0, 2]))

            # ---- |a| in place on DVE: fp16 abs == clear the sign bit, a
            # single-src tensor_scalar(bitwise_and 0x7fff) on the 4x-mode
            # path (vs 1x ACTIVATE(Abs) on ACT), ~0.8us per map ----
            for n in range(K2):
                au = absa[:, n].bitcast(mybir.dt.uint16)
                nc.vector.tensor_scalar(out=au, in0=au,
                                        scalar1=0x7FFF, scalar2=None,
                                        op0=mybir.AluOpType.bitwise_and)

            # ---- normalizer sum on the (otherwise idle) PE: 9
            # accumulating identity matmuls per (band, slot) PSUM bank ----
            acc = {}
            for bi, (t0, L) in enumerate(TILES):
                for s in range(NS):
                    acc[(bi, s)] = accp.tile([P, L], dt.float32, tag="acc",
                                             name=f"acc_{bi}_{s}")
            for n in range(K2):
                for bi, (t0, L) in enumerate(TILES):
                    for s in range(NS):
                        nc.tensor.matmul(out=acc[(bi, s)],
                                         lhsT=smat_t[:, 1, :],
                                         rhs=absa[:, n, s, t0:t0 + L],
                                         start=(n == 0), stop=(n == K2 - 1),
                                         skip_group_check=True)

            # ~51-ULP NR reciprocal straight off PSUM, then an ACT cast to
            # fp16 so the normalize muls run in 2x mode.
            def recip_band(bi):
                t0, L = TILES[bi]
                for s in range(NS):
                    nc.vector.reciprocal_approx_fast(
                        out=rcp32[:, s, t0:t0 + L], in_=acc[(bi, s)])
                for s in range(NS):
                    nc.scalar.copy(out=rcp[:, s, t0:t0 + L],
                                   in_=rcp32[:, s, t0:t0 + L])

            # ---- normalize + pre-shift weights into wsh ----
            # wsh[n][i, jd] = w_n[i-s, jd-cs]  (s = di-1 rows, cs = dj-1
            # cols).  With the (p = i//2, c = i%2) row packing, for each
            # row-shifted tap ONE output slot is a pure slot swap (free-dim
            # addressing -> direct DVE write) and only the other is a true
            # partition shift (PE shift-matmul + ACT evac):
            #   taps 0-2 (s=-1): c'=0 <- slot 1 direct;  c'=1 <- S_up@slot0
            #   taps 6-8 (s=+1): c'=1 <- slot 0 direct;  c'=0 <- S_dn@slot1
            # Band 0 builds first, then iteration 1's first TT is emitted,
            # then the rest of wsh builds in single remainder ops.
            ws_t = {n: wstagep.tile([P, W], dt.float16, tag=f"ws{n}",
                                    name=f"ws_{n}") for n in SHIFT_TAPS}
            # tap -> (stationary idx, matmul src slot, direct src slot)
            mm_plan = {0: (2, 0, 1), 1: (2, 0, 1), 2: (2, 0, 1),
                       6: (0, 1, 0), 7: (0, 1, 0), 8: (0, 1, 0)}

            def stage_cols(e0, e1):
                for n in SHIFT_TAPS:
                    c_src = mm_plan[n][1]
                    nc.vector.tensor_mul(ws_t[n][:, e0:e1],
                                         absa[:, n, c_src, e0:e1],
                                         rcp[:, c_src, e0:e1])

            def build_band(bi):
                t0, L = TILES[bi]
                for n in SHIFT_TAPS:  # S_up x3 then S_dn x3: 2 LDW / band
                    cs = n % 3 - 1
                    sidx = mm_plan[n][0]
                    c_out = 1 if n < 3 else 0
                    jd0, jd1 = max(0, cs), W + min(0, cs)
                    a0, a1 = max(jd0, t0), min(jd1, t0 + L)
                    psb = prepsum.tile([P, 512], dt.float32, tag="pps",
                                       name=f"pps_{n}_{bi}")
                    nc.tensor.matmul(out=psb[:, 0:a1 - a0],
                                     lhsT=smat_t[:, sidx, :],
                                     rhs=ws_t[n][:, a0 - cs:a1 - cs],
                                     start=True, stop=True,
                                     skip_group_check=True)
                    nc.scalar.copy(out=wsh[:, n, c_out, GU + a0:GU + a1],
                                   in_=psb[:, 0:a1 - a0])

            def direct_cols(e0, e1):
                # Slot-swap halves of the row-shifted taps, then the
                # unshifted middle row (both slots at once).  cs != 0 taps
                # read an odd column base (drops to 1x mode) but still beat
                # a matmul round-trip + evacuation.
                for n in SHIFT_TAPS:
                    cs = n % 3 - 1
                    c_dir = 0 if n < 3 else 1
                    c_src = mm_plan[n][2]
                    a0, a1 = max(max(0, cs), e0), min(W + min(0, cs), e1)
                    nc.vector.tensor_mul(
                        wsh[:, n, c_dir, GU + a0:GU + a1],
                        absa[:, n, c_src, a0 - cs:a1 - cs],
                        rcp[:, c_src, a0 - cs:a1 - cs])
                for n in (3, 5, 4):
                    cs = n % 3 - 1
                    a0, a1 = max(max(0, cs), e0), min(W + min(0, cs), e1)
                    nc.vector.tensor_mul(wsh[:, n, :, GU + a0:GU + a1],
                                         absa[:, n, :, a0 - cs:a1 - cs],
                                         rcp[:, :, a0 - cs:a1 - cs])

            # Band 0 first (its stage/direct read one column into band 1,
            # so band 1's recip runs up front too).
            recip_band(0)
            recip_band(1)
            stage_cols(0, 408)  # band 0 + the column build_band(1) reads
            direct_cols(0, TILES[0][1])
            build_band(0)
            recip_band(2)
            # Iteration 1, tile 0: starts while the rest of wsh builds.
            emit_tt(z0, f0, *TILES[0])
            stage_cols(408, W)
            direct_cols(TILES[0][1], W)
            build_band(1)
            emit_tt(z0, f0, *TILES[1])
            build_band(2)
            emit_tt(z0, f0, *TILES[2])

        psump = ctx.enter_context(tc.tile_pool(name="psum", bufs=8,
                                               space="PSUM"))
        loopp = ctx.enter_context(tc.tile_pool(name="loop", bufs=1))
        z1 = loopp.tile([P, K2, NS, WB], dt.float16, name="z1")
        zbufs = [z0, z1]
        # Final-iteration staging: evacuated per PSUM bank, DMA'd per bank.
        fout = loopp.tile([P, NS, W], dt.float32)

        # ---- stencil iterations ----
        # Matmuls for PSUM bank (c, tile t) are split by column dependency:
        # taps with dj<=1 only read z columns from tiles <= t, so they can
        # start as soon as the TT covering tile t lands; dj==2 taps read
        # one column of tile t+1.  This keeps PE busy throughout the DVE
        # phase (HAM stays at full clock).
        f_cur = f0
        for k in range(times):
            last = k == times - 1
            z = zbufs[k % 2]
            if k > 0:
                for (t0, L) in TT_TILES:
                    emit_tt(z, f_cur, t0, L)
            f_next = None if last else fpool.tile([P, NS, W], dt.float16,
                                                  tag="f")

            def emit_mms(ps, c, t0, L, entries, start, stop):
                # At the image's W edges the out-of-range tap column is
                # dropped (zero padding) instead of reading a z guard cell,
                # so z guards never need initializing.
                for mi, (n, c_src, sidx) in enumerate(entries):
                    dj = n % 3
                    m0 = GU + t0 + dj - 1
                    o0 = 1 if (t0 == 0 and dj == 0) else 0
                    o1 = L - 1 if (t0 + L == W and dj == 2) else L
                    nc.tensor.matmul(
                        out=ps[:, o0:o1], lhsT=smat_t[:, sidx, :],
                        rhs=z[:, n, c_src, m0 + o0:m0 + o1],
                        start=start and mi == 0,
                        stop=stop and mi == len(entries) - 1,
                        skip_group_check=True)

            pst = {}
            for ti, (t0, L) in enumerate(TILES):
                for c in (0, 1):
                    pst[(c, ti)] = psump.tile([P, L], dt.float32, tag="ps",
                                              name=f"ps_{c}_{ti}")

            def finish_bank(c, ti):
                # Last iteration: ACT evacuates the finished bank as fp32
                # and its DMA to DRAM fires immediately (per bank, so the
                # store pipelines with the remaining banks' matmuls);
                # otherwise ACT evacuates it as the next iteration's fp16
                # feature tile.
                p0, pl = TILES[ti]
                if last:
                    nc.scalar.copy(out=fout[:, c, p0:p0 + pl],
                                   in_=pst[(c, ti)])
                    nc.gpsimd.dma_start(out=out_r[:, c, p0:p0 + pl],
                                        in_=fout[:, c, p0:p0 + pl])
                else:
                    nc.scalar.copy(out=f_next[:, c, p0:p0 + pl],
                                   in_=pst[(c, ti)])

            # Phase ti: finish banks of tile ti-1 (dj2 taps + evacuate),
            # then open banks of tile ti (dj01 taps).
            for ti, (t0, L) in enumerate(TILES):
                if ti > 0:
                    p0, pl = TILES[ti - 1]
                    for c in (0, 1):
                        emit_mms(pst[(c, ti - 1)], c, p0, pl, dj2[c],
                                 start=False, stop=True)
                        finish_bank(c, ti - 1)
                for c in (0, 1):
                    emit_mms(pst[(c, ti)], c, t0, L,
                             dj01_t0[c] if ti == 0 else dj01[c],
                             start=True, stop=False)
            t0, L = TILES[-1]
            for c in (0, 1):
                emit_mms(pst[(c, 2)], c, t0, L, dj2[c], start=False,
                         stop=True)
                finish_bank(c, 2)
            f_cur = f_next

    nc._dbg = {'wsh': wsh, 'smat_t': smat_t}
    nc.finalize()
    return nc


def _get_program(times: int):
    if times not in _CACHE:
        _CACHE[times] = build_program(times)
    return _CACHE[times]


def _in_maps(affinity: np.ndarray, feature: np.ndarray):
    sm = _shift_mats()
    return [{
        # [9, 256, 1216] -> [128, 9, 2, 1216] fp16 (the SBUF layout)
        "aff": np.ascontiguousarray(
            affinity[b].astype(np.float16)
            .reshape(K2, P, NS, W).transpose(1, 0, 2, 3)),
        "feat": np.ascontiguousarray(feature[b].reshape(H, W),
                                     dtype=np.float16),
        "smat": sm,
    } for b in range(B)]


def _run(affinity, feature, times, trace=False):
    from concourse.bass_utils import run_bass_kernel_spmd

    nc = _get_program(int(times))
    res = run_bass_kernel_spmd(nc, _in_maps(affinity, feature),
                               core_ids=list(range(B)), trace=trace)
    outs = np.stack([np.asarray(res.results[b]["out"]) for b in range(B)])
    return outs.reshape(B, 1, H, W).astype(np.float32), res


def kernel(affinity, feature, times) -> np.ndarray:
    affinity = np.asarray(affinity)
    feature = np.asarray(feature)
    assert affinity.shape == (B, K2, H, W), affinity.shape
    assert feature.shape[0] == B and feature.shape[-2:] == (H, W)
    out, _ = _run(affinity, feature, int(times))
    return out


# revision 18
# speedup vs baseline: 1.3904x; 1.3904x over previous
"""AffinityPropagate Trainium2 kernel.

Problem: 24 iterations of a per-pixel-weighted 3x3 stencil (zero-padded)
on a [B=8, C=1, H=256, W=1216] image, weights = abs-normalized affinity
[B, 9, H, W].  Data-parallel over batch: one image per NeuronCore.

Per-core algorithm
------------------
Row i of the image maps to (partition p = i//2, slot c = i%2), so the
256 rows live on 128 partitions x 2 free-dim blocks.  With that mapping
a +-1 row shift never crosses a partition-chunk seam: it is a partition
shift by one (handled exactly by a 128x128 shifted-identity matmul,
zero-padding falls out of the missing matrix row) and/or a slot swap
(pure free-dim addressing).

Weights are normalized once, then pre-shifted so the per-iteration inner
loop is only:
  DVE : z[n]  = wsh[n] * f          (9 taps in one tensor_tensor via a
                                     stride-0 broadcast of f; fp16 -> 2x mode)
  PE  : out[c] += S_{n,c} @ z[n][shifted columns]   (PSUM fp32 accumulate)
  ACT : f' = cast(out)              (PSUM -> SBUF fp16 copy)
All column (W) shifts are plain free-dim address offsets into z's
guard-padded blocks; all row (H) shifts are the stationary matrices.

Preamble: inputs ship as fp16 in the SBUF-resident layout (half the HBM
traffic, fully contiguous DMA lines), |a| is a 4x-mode DVE
tensor_scalar(bitwise_and 0x7fff), the 9-map normalizer sum runs on the
otherwise-idle PE as accumulating identity matmuls, and the
normalize/pre-shift build is split band 0 vs rest with iteration 1's
first TT emitted in between, so the loop is running while the last two
thirds of wsh are still being built.
"""

import os
import sys

import numpy as np

for _p in ("/opt/trn_rl_repo", "/opt/pypackages"):
    if os.path.isdir(_p) and _p not in sys.path:
        sys.path.insert(0, _p)

B, K2, H, W = 8, 9, 256, 1216
P = 128          # partitions; row i -> (p=i//2, c=i%2)
NS = 2           # row slots per partition
GU = 2           # guard columns on each side of a z block (4B aligned)
WB = W + 2 * GU  # z/wsh block width
TILES = [(0, 406), (406, 406), (812, 404)]  # PSUM bank tiling (even sizes)
# TT tiling matches the PSUM tiling: iteration k+1's first TT needs only
# f_next tile 0, which ACT evacuates while tiles 1-2 of iteration k are
# still in flight.  (A coarser TT split serializes on that handoff.)
TT_TILES = TILES

# Per-bank matmul plan: out slot c' accumulates, for each tap n,
# z[n][src slot] routed through stationary matrix:
#   s = di-1 = -1: c'=0 <- (S_dn, slot 1),  c'=1 <- (I, slot 0)
#   s = 0        : c' <- (I, slot c')
#   s = +1       : c'=0 <- (I, slot 1),  c'=1 <- (S_up, slot 0)
# Stationary index: 0 = S_dn (k == m-1), 1 = I, 2 = S_up (k == m+1).
# Ordered so consecutive matmuls mostly share the stationary operand.
PLAN = {
    0: [(0, 1, 0), (1, 1, 0), (2, 1, 0),
        (3, 0, 1), (4, 0, 1), (5, 0, 1),
        (6, 1, 1), (7, 1, 1), (8, 1, 1)],
    1: [(0, 0, 1), (1, 0, 1), (2, 0, 1),
        (3, 1, 1), (4, 1, 1), (5, 1, 1),
        (6, 0, 2), (7, 0, 2), (8, 0, 2)],
}

# Taps with a row shift (di != 1).
SHIFT_TAPS = (0, 1, 2, 6, 7, 8)

_CACHE = {}


def _shift_mats() -> np.ndarray:
    """[3, 128, 128] fp16: S_dn (k==m-1), I, S_up (k==m+1) as lhsT[k, m]."""
    s = np.zeros((3, P, P), dtype=np.float16)
    k = np.arange(P - 1)
    s[0][k, k + 1] = 1.0      # out[m] = mov[m-1]
    s[1][np.arange(P), np.arange(P)] = 1.0
    s[2][k + 1, k] = 1.0      # out[m] = mov[m+1]
    return s


def build_program(times: int):
    import concourse.bacc as bacc
    import concourse.tile as tile
    from concourse import mybir
    from contextlib import ExitStack

    dt = mybir.dt
    nc = bacc.Bacc(trn_type="TRN2", target_bir_lowering=False, debug=False,
                   num_devices=B)

    # aff ships already transposed to the SBUF layout [p, n, c, j]: each
    # partition's slice is one contiguous DRAM run -> max DMA efficiency.
    aff = nc.dram_tensor("aff", [P, K2, NS, W], dt.float16,
                         kind="ExternalInput")
    feat = nc.dram_tensor("feat", [H, W], dt.float16, kind="ExternalInput")
    smat = nc.dram_tensor("smat", [3, P, P], dt.float16, kind="ExternalInput")
    out = nc.dram_tensor("out", [H, W], dt.float32, kind="ExternalOutput")

    feat_r = feat.ap().rearrange("(p c) j -> p c j", c=NS)
    out_r = out.ap().rearrange("(p c) j -> p c j", c=NS)

    with tile.TileContext(nc) as tc, ExitStack() as ctx:
        persist = ctx.enter_context(tc.tile_pool(name="persist", bufs=1))
        fpool = ctx.enter_context(tc.tile_pool(name="fpool", bufs=2))

        smat_t = persist.tile([P, 3, P], dt.float16)
        wsh = persist.tile([P, K2, NS, WB], dt.float16)
        # z buffer 0 lives outside the preamble pools so iteration 1's TT
        # does not wait for the preamble-pool release.  z guard columns are
        # never initialized: the edge matmuls clamp their column ranges
        # instead (the clamped-away taps are exactly the zero-padding).
        z0 = persist.tile([P, K2, NS, WB], dt.float16, name="z0")

        f0 = fpool.tile([P, NS, W], dt.float16, tag="f")

        # ---- loop emission helpers (iteration 1's TTs are interleaved
        # with the preamble build below) ----
        dj01 = {c: [e for e in PLAN[c] if e[0] % 3 <= 1] for c in (0, 1)}
        dj2 = {c: [e for e in PLAN[c] if e[0] % 3 == 2] for c in (0, 1)}
        # Tile 0 starts its accumulation group with a dj=1 tap (full column
        # range) so the edge-clamped dj=0 taps never own the start flag;
        # order keeps stationary runs contiguous (no extra LDWEIGHTS).
        tap_order = {0: [4, 3, 6, 7, 0, 1], 1: [4, 3, 0, 1, 6, 7]}
        dj01_t0 = {c: sorted(dj01[c], key=lambda e: tap_order[c].index(e[0]))
                   for c in (0, 1)}

        def emit_tt(z, f_cur, t0, L):
            fb = f_cur[:, :, t0:t0 + L].unsqueeze(1).broadcast_to(
                [P, K2, NS, L])
            nc.vector.tensor_tensor(
                out=z[:, :, :, GU + t0:GU + t0 + L],
                in0=wsh[:, :, :, GU + t0:GU + t0 + L],
                in1=fb, op=mybir.AluOpType.mult)

        with tc.tile_pool(name="pre", bufs=1) as prep, \
                tc.tile_pool(name="wstage", bufs=1) as wstagep, \
                tc.tile_pool(name="accps", bufs=6, space="PSUM") as accp, \
                tc.tile_pool(name="prepsum", bufs=2, space="PSUM") as prepsum:
            absa = prep.tile([P, K2, NS, W], dt.float16)
            rcp32 = prep.tile([P, NS, W], dt.float32)
            rcp = prep.tile([P, NS, W], dt.float16)

            # wsh cells the build never writes (guards + the one edge
            # column of each shifted tap) must be zero: they feed the loop
            # TT and the shift matmuls would accumulate garbage otherwise.
            nc.vector.memset(wsh[:, :, :, 0:GU + 1], 0.0)
            nc.vector.memset(wsh[:, :, :, GU + W - 1:WB], 0.0)

            # Affinity lands straight in absa: 4 DMAs so |a| can start
            # after the first two maps arrive.  Feature and the shift
            # matrices ride separate queues.
            batches = [(0, 2), (2, 2), (4, 2), (6, 3)]
            for n0, cnt in batches:
                nc.sync.dma_start(out=absa[:, n0:n0 + cnt],
                                  in_=aff.ap()[:, n0:n0 + cnt])
            nc.sync.dma_start(out=f0, in_=feat_r)
            nc.gpsimd.dma_start(out=smat_t, in_=smat.ap().transpose([1, 0, 2]))

            # ---- |a| in place on DVE: fp16 abs == clear the sign bit, a
            # single-src tensor_scalar(bitwise_and 0x7fff) on the 4x-mode
            # path (vs 1x ACTIVATE(Abs) on ACT), ~0.8us per map ----
            for n in range(K2):
                au = absa[:, n].bitcast(mybir.dt.uint16)
                nc.vector.tensor_scalar(out=au, in0=au,
                                        scalar1=0x7FFF, scalar2=None,
                                        op0=mybir.AluOpType.bitwise_and)

            # ---- normalizer sum on the (otherwise idle) PE: 9
            # accumulating identity matmuls per (band, slot) PSUM bank ----
            acc = {}
            for bi, (t0, L) in enumerate(TILES):
                for s in range(NS):
                    acc[(bi, s)] = accp.tile([P, L], dt.float32, tag="acc",
                                             name=f"acc_{bi}_{s}")
            for n in range(K2):
                for bi, (t0, L) in enumerate(TILES):
                    for s in range(NS):
                        nc.tensor.matmul(out=acc[(bi, s)],
                                         lhsT=smat_t[:, 1, :],
                                         rhs=absa[:, n, s, t0:t0 + L],
                                         start=(n == 0), stop=(n == K2 - 1),
                                         skip_group_check=True)

            # ~51-ULP NR reciprocal straight off PSUM, then an ACT cast to
            # fp16 so the normalize muls run in 2x mode.
            def recip_band(bi):
                t0, L = TILES[bi]
                for s in range(NS):
                    nc.vector.reciprocal_approx_fast(
                        out=rcp32[:, s, t0:t0 + L], in_=acc[(bi, s)])
                for s in range(NS):
                    nc.scalar.copy(out=rcp[:, s, t0:t0 + L],
                                   in_=rcp32[:, s, t0:t0 + L])

            # ---- normalize + pre-shift weights into wsh ----
            # wsh[n][i, jd] = w_n[i-s, jd-cs]  (s = di-1 rows, cs = dj-1
            # cols).  With the (p = i//2, c = i%2) row packing, for each
            # row-shifted tap ONE output slot is a pure slot swap (free-dim
            # addressing -> direct DVE write) and only the other is a true
            # partition shift (PE shift-matmul + ACT evac):
            #   taps 0-2 (s=-1): c'=0 <- slot 1 direct;  c'=1 <- S_up@slot0
            #   taps 6-8 (s=+1): c'=1 <- slot 0 direct;  c'=0 <- S_dn@slot1
            # Band 0 builds first, then iteration 1's first TT is emitted,
            # then the rest of wsh builds in single remainder ops.
            ws_t = {n: wstagep.tile([P, W], dt.float16, tag=f"ws{n}",
                                    name=f"ws_{n}") for n in SHIFT_TAPS}
            # tap -> (stationary idx, matmul src slot, direct src slot)
            mm_plan = {0: (2, 0, 1), 1: (2, 0, 1), 2: (2, 0, 1),
                       6: (0, 1, 0), 7: (0, 1, 0), 8: (0, 1, 0)}

            def stage_cols(e0, e1):
                for n in SHIFT_TAPS:
                    c_src = mm_plan[n][1]
                    nc.vector.tensor_mul(ws_t[n][:, e0:e1],
                                         absa[:, n, c_src, e0:e1],
                                         rcp[:, c_src, e0:e1])

            def build_band(bi):
                t0, L = TILES[bi]
                for n in SHIFT_TAPS:  # S_up x3 then S_dn x3: 2 LDW / band
                    cs = n % 3 - 1
                    sidx = mm_plan[n][0]
                    c_out = 1 if n < 3 else 0
                    jd0, jd1 = max(0, cs), W + min(0, cs)
                    a0, a1 = max(jd0, t0), min(jd1, t0 + L)
                    psb = prepsum.tile([P, 512], dt.float32, tag="pps",
                                       name=f"pps_{n}_{bi}")
                    nc.tensor.matmul(out=psb[:, 0:a1 - a0],
                                     lhsT=smat_t[:, sidx, :],
                                     rhs=ws_t[n][:, a0 - cs:a1 - cs],
                                     start=True, stop=True,
                                     skip_group_check=True)
                    nc.scalar.copy(out=wsh[:, n, c_out, GU + a0:GU + a1],
                                   in_=psb[:, 0:a1 - a0])

            def direct_cols(e0, e1):
                # Slot-swap halves of the row-shifted taps, then the
                # unshifted middle row (both slots at once).  cs != 0 taps
                # read an odd column base (drops to 1x mode) but still beat
                # a matmul round-trip + evacuation.
                for n in SHIFT_TAPS:
                    cs = n % 3 - 1
                    c_dir = 0 if n < 3 else 1
                    c_src = mm_plan[n][2]
                    a0, a1 = max(max(0, cs), e0), min(W + min(0, cs), e1)
                    nc.vector.tensor_mul(
                        wsh[:, n, c_dir, GU + a0:GU + a1],
                        absa[:, n, c_src, a0 - cs:a1 - cs],
                        rcp[:, c_src, a0 - cs:a1 - cs])
                for n in (3, 5, 4):
                    cs = n % 3 - 1
                    a0, a1 = max(max(0, cs), e0), min(W + min(0, cs), e1)
                    nc.vector.tensor_mul(wsh[:, n, :, GU + a0:GU + a1],
                                         absa[:, n, :, a0 - cs:a1 - cs],
                                         rcp[:, :, a0 - cs:a1 - cs])

            # Band 0 first (its stage/direct read one column into band 1,
            # so band 1's recip runs up front too).
            recip_band(0)
            recip_band(1)
            stage_cols(0, 408)  # band 0 + the column build_band(1) reads
            direct_cols(0, TILES[0][1])
            build_band(0)
            recip_band(2)
            # Iteration 1, tile 0: starts while the rest of wsh builds.
            emit_tt(z0, f0, *TILES[0])
            stage_cols(408, W)
            direct_cols(TILES[0][1], W)
            build_band(1)
            emit_tt(z0, f0, *TILES[1])
            build_band(2)
            emit_tt(z0, f0, *TILES[2])

        psump = ctx.enter_context(tc.tile_pool(name="psum", bufs=8,
                                               space="PSUM"))
        loopp = ctx.enter_context(tc.tile_pool(name="loop", bufs=1))
        z1 = loopp.tile([P, K2, NS, WB], dt.float16, name="z1")
        zbufs = [z0, z1]
        # Final-iteration staging: evacuated per PSUM bank, DMA'd per bank.
        fout = loopp.tile([P, NS, W], dt.float32)

        # ---- stencil iterations ----
        # Matmuls for PSUM bank (c, tile t) are split by column dependency:
        # taps with dj<=1 only read z columns from tiles <= t, so they can
        # start as soon as the TT covering tile t lands; dj==2 taps read
        # one column of tile t+1.  This keeps PE busy throughout the DVE
        # phase (HAM stays at full clock).
        f_cur = f0
        for k in range(times):
            last = k == times - 1
            z = zbufs[k % 2]
            if k > 0:
                for (t0, L) in TT_TILES:
                    emit_tt(z, f_cur, t0, L)
            f_next = None if last else fpool.tile([P, NS, W], dt.float16,
                                                  tag="f")

            def emit_mms(ps, c, t0, L, entries, start, stop):
                # At the image's W edges the out-of-range tap column is
                # dropped (zero padding) instead of reading a z guard cell,
                # so z guards never need initializing.
                for mi, (n, c_src, sidx) in enumerate(entries):
                    dj = n % 3
                    m0 = GU + t0 + dj - 1
                    o0 = 1 if (t0 == 0 and dj == 0) else 0
                    o1 = L - 1 if (t0 + L == W and dj == 2) else L
                    nc.tensor.matmul(
                        out=ps[:, o0:o1], lhsT=smat_t[:, sidx, :],
                        rhs=z[:, n, c_src, m0 + o0:m0 + o1],
                        start=start and mi == 0,
                        stop=stop and mi == len(entries) - 1,
                        skip_group_check=True)

            pst = {}
            for ti, (t0, L) in enumerate(TILES):
                for c in (0, 1):
                    pst[(c, ti)] = psump.tile([P, L], dt.float32, tag="ps",
                                              name=f"ps_{c}_{ti}")

            def finish_bank(c, ti):
                # Last iteration: ACT evacuates the finished bank as fp32
                # and its DMA to DRAM fires immediately (per bank, so the
                # store pipelines with the remaining banks' matmuls);
                # otherwise ACT evacuates it as the next iteration's fp16
                # feature tile.
                p0, pl = TILES[ti]
                if last:
                    nc.scalar.copy(out=fout[:, c, p0:p0 + pl],
                                   in_=pst[(c, ti)])
                    nc.gpsimd.dma_start(out=out_r[:, c, p0:p0 + pl],
                                        in_=fout[:, c, p0:p0 + pl])
                else:
                    nc.scalar.copy(out=f_next[:, c, p0:p0 + pl],
                                   in_=pst[(c, ti)])

            # Phase ti: finish banks of tile ti-1 (dj2 taps + evacuate),
            # then open banks of tile ti (dj01 taps).
            for ti, (t0, L) in enumerate(TILES):
                if ti > 0:
                    p0, pl = TILES[ti - 1]
                    for c in (0, 1):
                        emit_mms(pst[(c, ti - 1)], c, p0, pl, dj2[c],
                                 start=False, stop=True)
                        finish_bank(c, ti - 1)
                for c in (0, 1):
                    emit_mms(pst[(c, ti)], c, t0, L,
                             dj01_t0[c] if ti == 0 else dj01[c],
                             start=True, stop=False)
            t0, L = TILES[-1]
            for c in (0, 1):
                emit_mms(pst[(c, 2)], c, t0, L, dj2[c], start=False,
                         stop=True)
                finish_bank(c, 2)
            f_cur = f_next

    nc._dbg = {'wsh': wsh, 'smat_t': smat_t}
    nc.finalize()
    return nc


def _get_program(times: int):
    if times not in _CACHE:
        _CACHE[times] = build_program(times)
    return _CACHE[times]


def _in_maps(affinity: np.ndarray, feature: np.ndarray):
    sm = _shift_mats()
    return [{
        # [9, 256, 1216] -> [128, 9, 2, 1216] fp16 (the SBUF layout)
        "aff": np.ascontiguousarray(
            affinity[b].astype(np.float16)
            .reshape(K2, P, NS, W).transpose(1, 0, 2, 3)),
        "feat": np.ascontiguousarray(feature[b].reshape(H, W),
                                     dtype=np.float16),
        "smat": sm,
    } for b in range(B)]


def _run(affinity, feature, times, trace=False):
    from concourse.bass_utils import run_bass_kernel_spmd

    nc = _get_program(int(times))
    res = run_bass_kernel_spmd(nc, _in_maps(affinity, feature),
                               core_ids=list(range(B)), trace=trace)
    outs = np.stack([np.asarray(res.results[b]["out"]) for b in range(B)])
    return outs.reshape(B, 1, H, W).astype(np.float32), res


def kernel(affinity, feature, times) -> np.ndarray:
    affinity = np.asarray(affinity)
    feature = np.asarray(feature)
    assert affinity.shape == (B, K2, H, W), affinity.shape
    assert feature.shape[0] == B and feature.shape[-2:] == (H, W)
    out, _ = _run(affinity, feature, int(times))
    return out


# revision 19
# speedup vs baseline: 1.3954x; 1.0036x over previous
"""AffinityPropagate Trainium2 kernel.

Problem: 24 iterations of a per-pixel-weighted 3x3 stencil (zero-padded)
on a [B=8, C=1, H=256, W=1216] image, weights = abs-normalized affinity
[B, 9, H, W].  Data-parallel over batch: one image per NeuronCore.

Per-core algorithm
------------------
Row i of the image maps to (partition p = i//2, slot c = i%2), so the
256 rows live on 128 partitions x 2 free-dim blocks.  With that mapping
a +-1 row shift never crosses a partition-chunk seam: it is a partition
shift by one (handled exactly by a 128x128 shifted-identity matmul,
zero-padding falls out of the missing matrix row) and/or a slot swap
(pure free-dim addressing).

Weights are normalized once, then pre-shifted so the per-iteration inner
loop is only:
  DVE : z[n]  = wsh[n] * f          (9 taps in one tensor_tensor via a
                                     stride-0 broadcast of f; fp16 -> 2x mode)
  PE  : out[c] += S_{n,c} @ z[n][shifted columns]   (PSUM fp32 accumulate)
  ACT : f' = cast(out)              (PSUM -> SBUF fp16 copy)
All column (W) shifts are plain free-dim address offsets into z's
guard-padded blocks; all row (H) shifts are the stationary matrices.

Preamble: inputs ship as fp16 in the SBUF-resident layout (half the HBM
traffic, fully contiguous DMA lines), |a| is a 4x-mode DVE
tensor_scalar(bitwise_and 0x7fff), the 9-map normalizer sum runs on the
otherwise-idle PE as accumulating identity matmuls, and the
normalize/pre-shift build is split band 0 vs rest with iteration 1's
first TT emitted in between, so the loop is running while the last two
thirds of wsh are still being built.
"""

import os
import sys

import numpy as np

for _p in ("/opt/trn_rl_repo", "/opt/pypackages"):
    if os.path.isdir(_p) and _p not in sys.path:
        sys.path.insert(0, _p)

B, K2, H, W = 8, 9, 256, 1216
P = 128          # partitions; row i -> (p=i//2, c=i%2)
NS = 2           # row slots per partition
GU = 2           # guard columns on each side of a z block (4B aligned)
WB = W + 2 * GU  # z/wsh block width
TILES = [(0, 406), (406, 406), (812, 404)]  # PSUM bank tiling (even sizes)
# TT tiling matches the PSUM tiling: iteration k+1's first TT needs only
# f_next tile 0, which ACT evacuates while tiles 1-2 of iteration k are
# still in flight.  (A coarser TT split serializes on that handoff.)
TT_TILES = TILES

# Per-bank matmul plan: out slot c' accumulates, for each tap n,
# z[n][src slot] routed through stationary matrix:
#   s = di-1 = -1: c'=0 <- (S_dn, slot 1),  c'=1 <- (I, slot 0)
#   s = 0        : c' <- (I, slot c')
#   s = +1       : c'=0 <- (I, slot 1),  c'=1 <- (S_up, slot 0)
# Stationary index: 0 = S_dn (k == m-1), 1 = I, 2 = S_up (k == m+1).
# Ordered so consecutive matmuls mostly share the stationary operand.
PLAN = {
    0: [(0, 1, 0), (1, 1, 0), (2, 1, 0),
        (3, 0, 1), (4, 0, 1), (5, 0, 1),
        (6, 1, 1), (7, 1, 1), (8, 1, 1)],
    1: [(0, 0, 1), (1, 0, 1), (2, 0, 1),
        (3, 1, 1), (4, 1, 1), (5, 1, 1),
        (6, 0, 2), (7, 0, 2), (8, 0, 2)],
}

# Taps with a row shift (di != 1).
SHIFT_TAPS = (0, 1, 2, 6, 7, 8)

_CACHE = {}


def _shift_mats() -> np.ndarray:
    """[3, 128, 128] fp16: S_dn (k==m-1), I, S_up (k==m+1) as lhsT[k, m]."""
    s = np.zeros((3, P, P), dtype=np.float16)
    k = np.arange(P - 1)
    s[0][k, k + 1] = 1.0      # out[m] = mov[m-1]
    s[1][np.arange(P), np.arange(P)] = 1.0
    s[2][k + 1, k] = 1.0      # out[m] = mov[m+1]
    return s


def build_program(times: int):
    import concourse.bacc as bacc
    import concourse.tile as tile
    from concourse import mybir
    from contextlib import ExitStack

    dt = mybir.dt
    nc = bacc.Bacc(trn_type="TRN2", target_bir_lowering=False, debug=False,
                   num_devices=B)

    # aff ships already transposed to the SBUF layout [p, n, c, j]: each
    # partition's slice is one contiguous DRAM run -> max DMA efficiency.
    aff = nc.dram_tensor("aff", [P, K2, NS, W], dt.float16,
                         kind="ExternalInput")
    feat = nc.dram_tensor("feat", [H, W], dt.float16, kind="ExternalInput")
    smat = nc.dram_tensor("smat", [3, P, P], dt.float16, kind="ExternalInput")
    out = nc.dram_tensor("out", [H, W], dt.float16, kind="ExternalOutput")

    feat_r = feat.ap().rearrange("(p c) j -> p c j", c=NS)
    out_r = out.ap().rearrange("(p c) j -> p c j", c=NS)

    with tile.TileContext(nc) as tc, ExitStack() as ctx:
        persist = ctx.enter_context(tc.tile_pool(name="persist", bufs=1))
        fpool = ctx.enter_context(tc.tile_pool(name="fpool", bufs=2))

        smat_t = persist.tile([P, 3, P], dt.float16)
        wsh = persist.tile([P, K2, NS, WB], dt.float16)
        # z buffer 0 lives outside the preamble pools so iteration 1's TT
        # does not wait for the preamble-pool release.  z guard columns are
        # never initialized: the edge matmuls clamp their column ranges
        # instead (the clamped-away taps are exactly the zero-padding).
        z0 = persist.tile([P, K2, NS, WB], dt.float16, name="z0")

        f0 = fpool.tile([P, NS, W], dt.float16, tag="f")

        # ---- loop emission helpers (iteration 1's TTs are interleaved
        # with the preamble build below) ----
        dj01 = {c: [e for e in PLAN[c] if e[0] % 3 <= 1] for c in (0, 1)}
        dj2 = {c: [e for e in PLAN[c] if e[0] % 3 == 2] for c in (0, 1)}
        # Tile 0 starts its accumulation group with a dj=1 tap (full column
        # range) so the edge-clamped dj=0 taps never own the start flag;
        # order keeps stationary runs contiguous (no extra LDWEIGHTS).
        tap_order = {0: [4, 3, 6, 7, 0, 1], 1: [4, 3, 0, 1, 6, 7]}
        dj01_t0 = {c: sorted(dj01[c], key=lambda e: tap_order[c].index(e[0]))
                   for c in (0, 1)}

        def emit_tt(z, f_cur, t0, L):
            fb = f_cur[:, :, t0:t0 + L].unsqueeze(1).broadcast_to(
                [P, K2, NS, L])
            nc.vector.tensor_tensor(
                out=z[:, :, :, GU + t0:GU + t0 + L],
                in0=wsh[:, :, :, GU + t0:GU + t0 + L],
                in1=fb, op=mybir.AluOpType.mult)

        with tc.tile_pool(name="pre", bufs=1) as prep, \
                tc.tile_pool(name="wstage", bufs=1) as wstagep, \
                tc.tile_pool(name="accps", bufs=6, space="PSUM") as accp, \
                tc.tile_pool(name="prepsum", bufs=2, space="PSUM") as prepsum:
            absa = prep.tile([P, K2, NS, W], dt.float16)
            rcp32 = prep.tile([P, NS, W], dt.float32)
            rcp = prep.tile([P, NS, W], dt.float16)

            # wsh cells the build never writes (guards + the one edge
            # column of each shifted tap) must be zero: they feed the loop
            # TT and the shift matmuls would accumulate garbage otherwise.
            nc.vector.memset(wsh[:, :, :, 0:GU + 1], 0.0)
            nc.vector.memset(wsh[:, :, :, GU + W - 1:WB], 0.0)

            # Affinity lands straight in absa: 4 DMAs so |a| can start
            # after the first two maps arrive.  Feature and the shift
            # matrices ride separate queues.
            batches = [(0, 1), (1, 2), (3, 2), (5, 2), (7, 2)]
            for n0, cnt in batches:
                nc.sync.dma_start(out=absa[:, n0:n0 + cnt],
                                  in_=aff.ap()[:, n0:n0 + cnt])
            nc.sync.dma_start(out=f0, in_=feat_r)
            nc.gpsimd.dma_start(out=smat_t, in_=smat.ap().transpose([1, 0, 2]))

            # ---- |a| in place on DVE: fp16 abs == clear the sign bit, a
            # single-src tensor_scalar(bitwise_and 0x7fff) on the 4x-mode
            # path (vs 1x ACTIVATE(Abs) on ACT), ~0.8us per map ----
            for n in range(K2):
                au = absa[:, n].bitcast(mybir.dt.uint16)
                nc.vector.tensor_scalar(out=au, in0=au,
                                        scalar1=0x7FFF, scalar2=None,
                                        op0=mybir.AluOpType.bitwise_and)

            # ---- normalizer sum on the (otherwise idle) PE: 9
            # accumulating identity matmuls per (band, slot) PSUM bank ----
            acc = {}
            for bi, (t0, L) in enumerate(TILES):
                for s in range(NS):
                    acc[(bi, s)] = accp.tile([P, L], dt.float32, tag="acc",
                                             name=f"acc_{bi}_{s}")
            for n in range(K2):
                for bi, (t0, L) in enumerate(TILES):
                    for s in range(NS):
                        nc.tensor.matmul(out=acc[(bi, s)],
                                         lhsT=smat_t[:, 1, :],
                                         rhs=absa[:, n, s, t0:t0 + L],
                                         start=(n == 0), stop=(n == K2 - 1),
                                         skip_group_check=True)

            # ~51-ULP NR reciprocal straight off PSUM, then an ACT cast to
            # fp16 so the normalize muls run in 2x mode.
            def recip_band(bi):
                t0, L = TILES[bi]
                for s in range(NS):
                    nc.vector.reciprocal_approx_fast(
                        out=rcp32[:, s, t0:t0 + L], in_=acc[(bi, s)])
                for s in range(NS):
                    nc.scalar.copy(out=rcp[:, s, t0:t0 + L],
                                   in_=rcp32[:, s, t0:t0 + L])

            # ---- normalize + pre-shift weights into wsh ----
            # wsh[n][i, jd] = w_n[i-s, jd-cs]  (s = di-1 rows, cs = dj-1
            # cols).  With the (p = i//2, c = i%2) row packing, for each
            # row-shifted tap ONE output slot is a pure slot swap (free-dim
            # addressing -> direct DVE write) and only the other is a true
            # partition shift (PE shift-matmul + ACT evac):
            #   taps 0-2 (s=-1): c'=0 <- slot 1 direct;  c'=1 <- S_up@slot0
            #   taps 6-8 (s=+1): c'=1 <- slot 0 direct;  c'=0 <- S_dn@slot1
            # Band 0 builds first, then iteration 1's first TT is emitted,
            # then the rest of wsh builds in single remainder ops.
            ws03 = wstagep.tile([P, 3, W], dt.float16, name="ws03")
            ws68 = wstagep.tile([P, 3, W], dt.float16, name="ws68")
            ws_t = {0: ws03[:, 0], 1: ws03[:, 1], 2: ws03[:, 2],
                    6: ws68[:, 0], 7: ws68[:, 1], 8: ws68[:, 2]}
            # tap -> (stationary idx, matmul src slot, direct src slot)
            mm_plan = {0: (2, 0, 1), 1: (2, 0, 1), 2: (2, 0, 1),
                       6: (0, 1, 0), 7: (0, 1, 0), 8: (0, 1, 0)}

            def stage_cols(e0, e1):
                # One TT per tap-triple: the three maps ride the n axis,
                # the (per-slot) rcp broadcasts across it with stride 0.
                for wst, n0, c_src in ((ws03, 0, 0), (ws68, 6, 1)):
                    rb = rcp[:, c_src, e0:e1].unsqueeze(1).broadcast_to(
                        [P, 3, e1 - e0])
                    nc.vector.tensor_tensor(
                        out=wst[:, :, e0:e1],
                        in0=absa[:, n0:n0 + 3, c_src, e0:e1],
                        in1=rb, op=mybir.AluOpType.mult)

            def build_band(bi):
                t0, L = TILES[bi]
                for n in SHIFT_TAPS:  # S_up x3 then S_dn x3: 2 LDW / band
                    cs = n % 3 - 1
                    sidx = mm_plan[n][0]
                    c_out = 1 if n < 3 else 0
                    jd0, jd1 = max(0, cs), W + min(0, cs)
                    a0, a1 = max(jd0, t0), min(jd1, t0 + L)
                    psb = prepsum.tile([P, 512], dt.float32, tag="pps",
                                       name=f"pps_{n}_{bi}")
                    nc.tensor.matmul(out=psb[:, 0:a1 - a0],
                                     lhsT=smat_t[:, sidx, :],
                                     rhs=ws_t[n][:, a0 - cs:a1 - cs],
                                     start=True, stop=True,
                                     skip_group_check=True)
                    nc.scalar.copy(out=wsh[:, n, c_out, GU + a0:GU + a1],
                                   in_=psb[:, 0:a1 - a0])

            def direct_cols(e0, e1):
                # Slot-swap halves of the row-shifted taps, then the
                # unshifted middle row (both slots at once).  cs != 0 taps
                # read an odd column base (drops to 1x mode) but still beat
                # a matmul round-trip + evacuation.
                for n in SHIFT_TAPS:
                    cs = n % 3 - 1
                    c_dir = 0 if n < 3 else 1
                    c_src = mm_plan[n][2]
                    a0, a1 = max(max(0, cs), e0), min(W + min(0, cs), e1)
                    nc.vector.tensor_mul(
                        wsh[:, n, c_dir, GU + a0:GU + a1],
                        absa[:, n, c_src, a0 - cs:a1 - cs],
                        rcp[:, c_src, a0 - cs:a1 - cs])
                for n in (3, 5, 4):
                    cs = n % 3 - 1
                    a0, a1 = max(max(0, cs), e0), min(W + min(0, cs), e1)
                    nc.vector.tensor_mul(wsh[:, n, :, GU + a0:GU + a1],
                                         absa[:, n, :, a0 - cs:a1 - cs],
                                         rcp[:, :, a0 - cs:a1 - cs])

            # Band 0 first (its stage/direct read one column into band 1,
            # so band 1's recip runs up front too).
            recip_band(0)
            recip_band(1)
            stage_cols(0, 408)  # band 0 + the column build_band(1) reads
            direct_cols(0, TILES[0][1])
            build_band(0)
            recip_band(2)
            # Iteration 1, tile 0: starts while the rest of wsh builds.
            emit_tt(z0, f0, *TILES[0])
            stage_cols(408, W)
            direct_cols(TILES[0][1], W)
            build_band(1)
            emit_tt(z0, f0, *TILES[1])
            build_band(2)
            emit_tt(z0, f0, *TILES[2])

        psump = ctx.enter_context(tc.tile_pool(name="psum", bufs=8,
                                               space="PSUM"))
        loopp = ctx.enter_context(tc.tile_pool(name="loop", bufs=1))
        z1 = loopp.tile([P, K2, NS, WB], dt.float16, name="z1")
        zbufs = [z0, z1]
        # Final-iteration staging: evacuated per PSUM bank, DMA'd per bank.
        fout = loopp.tile([P, NS, W], dt.float16)

        # ---- stencil iterations ----
        # Matmuls for PSUM bank (c, tile t) are split by column dependency:
        # taps with dj<=1 only read z columns from tiles <= t, so they can
        # start as soon as the TT covering tile t lands; dj==2 taps read
        # one column of tile t+1.  This keeps PE busy throughout the DVE
        # phase (HAM stays at full clock).
        f_cur = f0
        for k in range(times):
            last = k == times - 1
            z = zbufs[k % 2]
            if k > 0:
                for (t0, L) in TT_TILES:
                    emit_tt(z, f_cur, t0, L)
            f_next = None if last else fpool.tile([P, NS, W], dt.float16,
                                                  tag="f")

            def emit_mms(ps, c, t0, L, entries, start, stop):
                # At the image's W edges the out-of-range tap column is
                # dropped (zero padding) instead of reading a z guard cell,
                # so z guards never need initializing.
                for mi, (n, c_src, sidx) in enumerate(entries):
                    dj = n % 3
                    m0 = GU + t0 + dj - 1
                    o0 = 1 if (t0 == 0 and dj == 0) else 0
                    o1 = L - 1 if (t0 + L == W and dj == 2) else L
                    nc.tensor.matmul(
                        out=ps[:, o0:o1], lhsT=smat_t[:, sidx, :],
                        rhs=z[:, n, c_src, m0 + o0:m0 + o1],
                        start=start and mi == 0,
                        stop=stop and mi == len(entries) - 1,
                        skip_group_check=True)

            pst = {}
            for ti, (t0, L) in enumerate(TILES):
                for c in (0, 1):
                    pst[(c, ti)] = psump.tile([P, L], dt.float32, tag="ps",
                                              name=f"ps_{c}_{ti}")

            def finish_bank(c, ti):
                # Last iteration: ACT evacuates the finished bank as fp32
                # and its DMA to DRAM fires immediately (per bank, so the
                # store pipelines with the remaining banks' matmuls);
                # otherwise ACT evacuates it as the next iteration's fp16
                # feature tile.
                p0, pl = TILES[ti]
                if last:
                    nc.scalar.copy(out=fout[:, c, p0:p0 + pl],
                                   in_=pst[(c, ti)])
                    nc.gpsimd.dma_start(out=out_r[:, c, p0:p0 + pl],
                                        in_=fout[:, c, p0:p0 + pl])
                else:
                    nc.scalar.copy(out=f_next[:, c, p0:p0 + pl],
                                   in_=pst[(c, ti)])

            # Phase ti: finish banks of tile ti-1 (dj2 taps + evacuate),
            # then open banks of tile ti (dj01 taps).
            for ti, (t0, L) in enumerate(TILES):
                if ti > 0:
                    p0, pl = TILES[ti - 1]
                    for c in (0, 1):
                        emit_mms(pst[(c, ti - 1)], c, p0, pl, dj2[c],
                                 start=False, stop=True)
                        finish_bank(c, ti - 1)
                for c in (0, 1):
                    emit_mms(pst[(c, ti)], c, t0, L,
                             dj01_t0[c] if ti == 0 else dj01[c],
                             start=True, stop=False)
            t0, L = TILES[-1]
            for c in (0, 1):
                emit_mms(pst[(c, 2)], c, t0, L, dj2[c], start=False,
                         stop=True)
                finish_bank(c, 2)
            f_cur = f_next

    nc._dbg = {'wsh': wsh, 'smat_t': smat_t}
    nc.finalize()
    return nc


def _get_program(times: int):
    if times not in _CACHE:
        _CACHE[times] = build_program(times)
    return _CACHE[times]


def _in_maps(affinity: np.ndarray, feature: np.ndarray):
    sm = _shift_mats()
    return [{
        # [9, 256, 1216] -> [128, 9, 2, 1216] fp16 (the SBUF layout)
        "aff": np.ascontiguousarray(
            affinity[b].astype(np.float16)
            .reshape(K2, P, NS, W).transpose(1, 0, 2, 3)),
        "feat": np.ascontiguousarray(feature[b].reshape(H, W),
                                     dtype=np.float16),
        "smat": sm,
    } for b in range(B)]


def _run(affinity, feature, times, trace=False):
    from concourse.bass_utils import run_bass_kernel_spmd

    nc = _get_program(int(times))
    res = run_bass_kernel_spmd(nc, _in_maps(affinity, feature),
                               core_ids=list(range(B)), trace=trace)
    outs = np.stack([np.asarray(res.results[b]["out"]) for b in range(B)])
    return outs.reshape(B, 1, H, W).astype(np.float32), res


def kernel(affinity, feature, times) -> np.ndarray:
    affinity = np.asarray(affinity)
    feature = np.asarray(feature)
    assert affinity.shape == (B, K2, H, W), affinity.shape
    assert feature.shape[0] == B and feature.shape[-2:] == (H, W)
    out, _ = _run(affinity, feature, int(times))
    return out


# revision 20
# speedup vs baseline: 1.3992x; 1.0027x over previous
"""AffinityPropagate Trainium2 kernel.

Problem: 24 iterations of a per-pixel-weighted 3x3 stencil (zero-padded)
on a [B=8, C=1, H=256, W=1216] image, weights = abs-normalized affinity
[B, 9, H, W].  Data-parallel over batch: one image per NeuronCore.

Per-core algorithm
------------------
Row i of the image maps to (partition p = i//2, slot c = i%2), so the
256 rows live on 128 partitions x 2 free-dim blocks.  With that mapping
a +-1 row shift never crosses a partition-chunk seam: it is a partition
shift by one (handled exactly by a 128x128 shifted-identity matmul,
zero-padding falls out of the missing matrix row) and/or a slot swap
(pure free-dim addressing).

Weights are normalized once, then pre-shifted so the per-iteration inner
loop is only:
  DVE : z[n]  = wsh[n] * f          (9 taps in one tensor_tensor via a
                                     stride-0 broadcast of f; fp16 -> 2x mode)
  PE  : out[c] += S_{n,c} @ z[n][shifted columns]   (PSUM fp32 accumulate)
  ACT : f' = cast(out)              (PSUM -> SBUF fp16 copy)
All column (W) shifts are plain free-dim address offsets into z's
guard-padded blocks; all row (H) shifts are the stationary matrices.

Preamble: inputs ship as fp16 in the SBUF-resident layout (half the HBM
traffic, fully contiguous DMA lines), |a| is a 4x-mode DVE
tensor_scalar(bitwise_and 0x7fff), the 9-map normalizer sum runs on the
otherwise-idle PE as accumulating identity matmuls, and the
normalize/pre-shift build is split band 0 vs rest with iteration 1's
first TT emitted in between, so the loop is running while the last two
thirds of wsh are still being built.
"""

import os
import sys

import numpy as np

for _p in ("/opt/trn_rl_repo", "/opt/pypackages"):
    if os.path.isdir(_p) and _p not in sys.path:
        sys.path.insert(0, _p)

B, K2, H, W = 8, 9, 256, 1216
P = 128          # partitions; row i -> (p=i//2, c=i%2)
NS = 2           # row slots per partition
GU = 2           # guard columns on each side of a z block (4B aligned)
WB = W + 2 * GU  # z/wsh block width
TILES = [(0, 406), (406, 406), (812, 404)]  # PSUM bank tiling (even sizes)
# TT tiling matches the PSUM tiling: iteration k+1's first TT needs only
# f_next tile 0, which ACT evacuates while tiles 1-2 of iteration k are
# still in flight.  (A coarser TT split serializes on that handoff.)
TT_TILES = TILES

# Per-bank matmul plan: out slot c' accumulates, for each tap n,
# z[n][src slot] routed through stationary matrix:
#   s = di-1 = -1: c'=0 <- (S_dn, slot 1),  c'=1 <- (I, slot 0)
#   s = 0        : c' <- (I, slot c')
#   s = +1       : c'=0 <- (I, slot 1),  c'=1 <- (S_up, slot 0)
# Stationary index: 0 = S_dn (k == m-1), 1 = I, 2 = S_up (k == m+1).
# Ordered so consecutive matmuls mostly share the stationary operand.
PLAN = {
    0: [(0, 1, 0), (1, 1, 0), (2, 1, 0),
        (3, 0, 1), (4, 0, 1), (5, 0, 1),
        (6, 1, 1), (7, 1, 1), (8, 1, 1)],
    1: [(0, 0, 1), (1, 0, 1), (2, 0, 1),
        (3, 1, 1), (4, 1, 1), (5, 1, 1),
        (6, 0, 2), (7, 0, 2), (8, 0, 2)],
}

# Taps with a row shift (di != 1).
SHIFT_TAPS = (0, 1, 2, 6, 7, 8)

_CACHE = {}


def _shift_mats() -> np.ndarray:
    """[3, 128, 128] fp16: S_dn (k==m-1), I, S_up (k==m+1) as lhsT[k, m]."""
    s = np.zeros((3, P, P), dtype=np.float16)
    k = np.arange(P - 1)
    s[0][k, k + 1] = 1.0      # out[m] = mov[m-1]
    s[1][np.arange(P), np.arange(P)] = 1.0
    s[2][k + 1, k] = 1.0      # out[m] = mov[m+1]
    return s


def build_program(times: int):
    import concourse.bacc as bacc
    import concourse.tile as tile
    from concourse import mybir
    from contextlib import ExitStack

    dt = mybir.dt
    nc = bacc.Bacc(trn_type="TRN2", target_bir_lowering=False, debug=False,
                   num_devices=B)

    # aff ships already transposed to the SBUF layout [p, n, c, j]: each
    # partition's slice is one contiguous DRAM run -> max DMA efficiency.
    aff = nc.dram_tensor("aff", [P, K2, NS, W], dt.float16,
                         kind="ExternalInput")
    feat = nc.dram_tensor("feat", [H, W], dt.float16, kind="ExternalInput")
    smat = nc.dram_tensor("smat", [3, P, P], dt.float16, kind="ExternalInput")
    out = nc.dram_tensor("out", [H, W], dt.float16, kind="ExternalOutput")

    feat_r = feat.ap().rearrange("(p c) j -> p c j", c=NS)
    out_r = out.ap().rearrange("(p c) j -> p c j", c=NS)

    with tile.TileContext(nc) as tc, ExitStack() as ctx:
        persist = ctx.enter_context(tc.tile_pool(name="persist", bufs=1))
        fpool = ctx.enter_context(tc.tile_pool(name="fpool", bufs=2))

        smat_t = persist.tile([P, 3, P], dt.float16)
        wsh = persist.tile([P, K2, NS, WB], dt.float16)
        # z buffer 0 lives outside the preamble pools so iteration 1's TT
        # does not wait for the preamble-pool release.  z guard columns are
        # never initialized: the edge matmuls clamp their column ranges
        # instead (the clamped-away taps are exactly the zero-padding).
        z0 = persist.tile([P, K2, NS, WB], dt.float16, name="z0")

        f0 = fpool.tile([P, NS, W], dt.float16, tag="f")

        # ---- loop emission helpers (iteration 1's TTs are interleaved
        # with the preamble build below) ----
        dj01 = {c: [e for e in PLAN[c] if e[0] % 3 <= 1] for c in (0, 1)}
        dj2 = {c: [e for e in PLAN[c] if e[0] % 3 == 2] for c in (0, 1)}
        # Tile 0 starts its accumulation group with a dj=1 tap (full column
        # range) so the edge-clamped dj=0 taps never own the start flag;
        # order keeps stationary runs contiguous (no extra LDWEIGHTS).
        tap_order = {0: [4, 3, 6, 7, 0, 1], 1: [4, 3, 0, 1, 6, 7]}
        dj01_t0 = {c: sorted(dj01[c], key=lambda e: tap_order[c].index(e[0]))
                   for c in (0, 1)}

        def emit_tt(z, f_cur, t0, L):
            fb = f_cur[:, :, t0:t0 + L].unsqueeze(1).broadcast_to(
                [P, K2, NS, L])
            nc.vector.tensor_tensor(
                out=z[:, :, :, GU + t0:GU + t0 + L],
                in0=wsh[:, :, :, GU + t0:GU + t0 + L],
                in1=fb, op=mybir.AluOpType.mult)

        with tc.tile_pool(name="pre", bufs=1) as prep, \
                tc.tile_pool(name="wstage", bufs=1) as wstagep, \
                tc.tile_pool(name="accps", bufs=6, space="PSUM") as accp, \
                tc.tile_pool(name="prepsum", bufs=2, space="PSUM") as prepsum:
            absa = prep.tile([P, K2, NS, W], dt.float16)
            rcp32 = prep.tile([P, NS, W], dt.float32)
            rcp = prep.tile([P, NS, W], dt.float16)

            # wsh cells the build never writes (guards + the one edge
            # column of each shifted tap) must be zero: they feed the loop
            # TT and the shift matmuls would accumulate garbage otherwise.
            nc.vector.memset(wsh[:, :, :, 0:GU + 1], 0.0)
            nc.vector.memset(wsh[:, :, :, GU + W - 1:WB], 0.0)

            # Affinity lands straight in absa: 4 DMAs so |a| can start
            # after the first two maps arrive.  Feature and the shift
            # matrices ride separate queues.
            batches = [(0, 1), (1, 2), (3, 2), (5, 2), (7, 2)]
            for i, (n0, cnt) in enumerate(batches):
                # Alternate trigger engines: descriptor issue is ~600ns
                # serial per engine, so splitting across Sync and Scalar
                # gets the queues filled twice as fast.
                eng = nc.sync if i % 2 == 0 else nc.scalar
                eng.dma_start(out=absa[:, n0:n0 + cnt],
                              in_=aff.ap()[:, n0:n0 + cnt])
            nc.scalar.dma_start(out=f0, in_=feat_r)
            nc.gpsimd.dma_start(out=smat_t, in_=smat.ap().transpose([1, 0, 2]))

            # ---- |a| in place on DVE: fp16 abs == clear the sign bit, a
            # single-src tensor_scalar(bitwise_and 0x7fff) on the 4x-mode
            # path (vs 1x ACTIVATE(Abs) on ACT), ~0.8us per map ----
            for n in range(K2):
                au = absa[:, n].bitcast(mybir.dt.uint16)
                nc.vector.tensor_scalar(out=au, in0=au,
                                        scalar1=0x7FFF, scalar2=None,
                                        op0=mybir.AluOpType.bitwise_and)

            # ---- normalizer sum on the (otherwise idle) PE: 9
            # accumulating identity matmuls per (band, slot) PSUM bank ----
            acc = {}
            for bi, (t0, L) in enumerate(TILES):
                for s in range(NS):
                    acc[(bi, s)] = accp.tile([P, L], dt.float32, tag="acc",
                                             name=f"acc_{bi}_{s}")
            for n in range(K2):
                for bi, (t0, L) in enumerate(TILES):
                    for s in range(NS):
                        nc.tensor.matmul(out=acc[(bi, s)],
                                         lhsT=smat_t[:, 1, :],
                                         rhs=absa[:, n, s, t0:t0 + L],
                                         start=(n == 0), stop=(n == K2 - 1),
                                         skip_group_check=True)

            # ~51-ULP NR reciprocal straight off PSUM, then an ACT cast to
            # fp16 so the normalize muls run in 2x mode.
            def recip_band(bi):
                t0, L = TILES[bi]
                for s in range(NS):
                    nc.vector.reciprocal_approx_fast(
                        out=rcp32[:, s, t0:t0 + L], in_=acc[(bi, s)])
                for s in range(NS):
                    nc.scalar.copy(out=rcp[:, s, t0:t0 + L],
                                   in_=rcp32[:, s, t0:t0 + L])

            # ---- normalize + pre-shift weights into wsh ----
            # wsh[n][i, jd] = w_n[i-s, jd-cs]  (s = di-1 rows, cs = dj-1
            # cols).  With the (p = i//2, c = i%2) row packing, for each
            # row-shifted tap ONE output slot is a pure slot swap (free-dim
            # addressing -> direct DVE write) and only the other is a true
            # partition shift (PE shift-matmul + ACT evac):
            #   taps 0-2 (s=-1): c'=0 <- slot 1 direct;  c'=1 <- S_up@slot0
            #   taps 6-8 (s=+1): c'=1 <- slot 0 direct;  c'=0 <- S_dn@slot1
            # Band 0 builds first, then iteration 1's first TT is emitted,
            # then the rest of wsh builds in single remainder ops.
            ws03 = wstagep.tile([P, 3, W], dt.float16, name="ws03")
            ws68 = wstagep.tile([P, 3, W], dt.float16, name="ws68")
            ws_t = {0: ws03[:, 0], 1: ws03[:, 1], 2: ws03[:, 2],
                    6: ws68[:, 0], 7: ws68[:, 1], 8: ws68[:, 2]}
            # tap -> (stationary idx, matmul src slot, direct src slot)
            mm_plan = {0: (2, 0, 1), 1: (2, 0, 1), 2: (2, 0, 1),
                       6: (0, 1, 0), 7: (0, 1, 0), 8: (0, 1, 0)}

            def stage_cols(e0, e1):
                # One TT per tap-triple: the three maps ride the n axis,
                # the (per-slot) rcp broadcasts across it with stride 0.
                for wst, n0, c_src in ((ws03, 0, 0), (ws68, 6, 1)):
                    rb = rcp[:, c_src, e0:e1].unsqueeze(1).broadcast_to(
                        [P, 3, e1 - e0])
                    nc.vector.tensor_tensor(
                        out=wst[:, :, e0:e1],
                        in0=absa[:, n0:n0 + 3, c_src, e0:e1],
                        in1=rb, op=mybir.AluOpType.mult)

            def build_band(bi):
                t0, L = TILES[bi]
                for n in SHIFT_TAPS:  # S_up x3 then S_dn x3: 2 LDW / band
                    cs = n % 3 - 1
                    sidx = mm_plan[n][0]
                    c_out = 1 if n < 3 else 0
                    jd0, jd1 = max(0, cs), W + min(0, cs)
                    a0, a1 = max(jd0, t0), min(jd1, t0 + L)
                    psb = prepsum.tile([P, 512], dt.float32, tag="pps",
                                       name=f"pps_{n}_{bi}")
                    nc.tensor.matmul(out=psb[:, 0:a1 - a0],
                                     lhsT=smat_t[:, sidx, :],
                                     rhs=ws_t[n][:, a0 - cs:a1 - cs],
                                     start=True, stop=True,
                                     skip_group_check=True)
                    nc.scalar.copy(out=wsh[:, n, c_out, GU + a0:GU + a1],
                                   in_=psb[:, 0:a1 - a0])

            def direct_cols(e0, e1):
                # Slot-swap halves of the row-shifted taps, then the
                # unshifted middle row (both slots at once).  cs != 0 taps
                # read an odd column base (drops to 1x mode) but still beat
                # a matmul round-trip + evacuation.
                for n in SHIFT_TAPS:
                    cs = n % 3 - 1
                    c_dir = 0 if n < 3 else 1
                    c_src = mm_plan[n][2]
                    a0, a1 = max(max(0, cs), e0), min(W + min(0, cs), e1)
                    nc.vector.tensor_mul(
                        wsh[:, n, c_dir, GU + a0:GU + a1],
                        absa[:, n, c_src, a0 - cs:a1 - cs],
                        rcp[:, c_src, a0 - cs:a1 - cs])
                for n in (3, 5, 4):
                    cs = n % 3 - 1
                    a0, a1 = max(max(0, cs), e0), min(W + min(0, cs), e1)
                    nc.vector.tensor_mul(wsh[:, n, :, GU + a0:GU + a1],
                                         absa[:, n, :, a0 - cs:a1 - cs],
                                         rcp[:, :, a0 - cs:a1 - cs])

            # Band 0 first (its stage/direct read one column into band 1,
            # so band 1's recip runs up front too).
            recip_band(0)
            recip_band(1)
            stage_cols(0, 408)  # band 0 + the column build_band(1) reads
            direct_cols(0, TILES[0][1])
            build_band(0)
            recip_band(2)
            # Iteration 1, tile 0: starts while the rest of wsh builds.
            emit_tt(z0, f0, *TILES[0])
            stage_cols(408, W)
            direct_cols(TILES[0][1], W)
            build_band(1)
            emit_tt(z0, f0, *TILES[1])
            build_band(2)
            emit_tt(z0, f0, *TILES[2])

        psump = ctx.enter_context(tc.tile_pool(name="psum", bufs=8,
                                               space="PSUM"))
        loopp = ctx.enter_context(tc.tile_pool(name="loop", bufs=1))
        z1 = loopp.tile([P, K2, NS, WB], dt.float16, name="z1")
        zbufs = [z0, z1]
        # Final-iteration staging: evacuated per PSUM bank, DMA'd per bank.
        fout = loopp.tile([P, NS, W], dt.float16)

        # ---- stencil iterations ----
        # Matmuls for PSUM bank (c, tile t) are split by column dependency:
        # taps with dj<=1 only read z columns from tiles <= t, so they can
        # start as soon as the TT covering tile t lands; dj==2 taps read
        # one column of tile t+1.  This keeps PE busy throughout the DVE
        # phase (HAM stays at full clock).
        f_cur = f0
        for k in range(times):
            last = k == times - 1
            z = zbufs[k % 2]
            if k > 0:
                for (t0, L) in TT_TILES:
                    emit_tt(z, f_cur, t0, L)
            f_next = None if last else fpool.tile([P, NS, W], dt.float16,
                                                  tag="f")

            def emit_mms(ps, c, t0, L, entries, start, stop):
                # At the image's W edges the out-of-range tap column is
                # dropped (zero padding) instead of reading a z guard cell,
                # so z guards never need initializing.
                for mi, (n, c_src, sidx) in enumerate(entries):
                    dj = n % 3
                    m0 = GU + t0 + dj - 1
                    o0 = 1 if (t0 == 0 and dj == 0) else 0
                    o1 = L - 1 if (t0 + L == W and dj == 2) else L
                    nc.tensor.matmul(
                        out=ps[:, o0:o1], lhsT=smat_t[:, sidx, :],
                        rhs=z[:, n, c_src, m0 + o0:m0 + o1],
                        start=start and mi == 0,
                        stop=stop and mi == len(entries) - 1,
                        skip_group_check=True)

            pst = {}
            for ti, (t0, L) in enumerate(TILES):
                for c in (0, 1):
                    pst[(c, ti)] = psump.tile([P, L], dt.float32, tag="ps",
                                              name=f"ps_{c}_{ti}")

            def finish_bank(c, ti):
                # Last iteration: ACT evacuates the finished bank as fp32
                # and its DMA to DRAM fires immediately (per bank, so the
                # store pipelines with the remaining banks' matmuls);
                # otherwise ACT evacuates it as the next iteration's fp16
                # feature tile.
                p0, pl = TILES[ti]
                if last:
                    nc.scalar.copy(out=fout[:, c, p0:p0 + pl],
                                   in_=pst[(c, ti)])
                    nc.gpsimd.dma_start(out=out_r[:, c, p0:p0 + pl],
                                        in_=fout[:, c, p0:p0 + pl])
                else:
                    nc.scalar.copy(out=f_next[:, c, p0:p0 + pl],
                                   in_=pst[(c, ti)])

            # Phase ti: finish banks of tile ti-1 (dj2 taps + evacuate),
            # then open banks of tile ti (dj01 taps).
            for ti, (t0, L) in enumerate(TILES):
                if ti > 0:
                    p0, pl = TILES[ti - 1]
                    for c in (0, 1):
                        emit_mms(pst[(c, ti - 1)], c, p0, pl, dj2[c],
                                 start=False, stop=True)
                        finish_bank(c, ti - 1)
                for c in (0, 1):
                    emit_mms(pst[(c, ti)], c, t0, L,
                             dj01_t0[c] if ti == 0 else dj01[c],
                             start=True, stop=False)
            t0, L = TILES[-1]
            for c in (0, 1):
                emit_mms(pst[(c, 2)], c, t0, L, dj2[c], start=False,
                         stop=True)
                finish_bank(c, 2)
            f_cur = f_next

    nc._dbg = {'wsh': wsh, 'smat_t': smat_t}
    nc.finalize()
    return nc


def _get_program(times: int):
    if times not in _CACHE:
        _CACHE[times] = build_program(times)
    return _CACHE[times]


def _in_maps(affinity: np.ndarray, feature: np.ndarray):
    sm = _shift_mats()
    return [{
        # [9, 256, 1216] -> [128, 9, 2, 1216] fp16 (the SBUF layout)
        "aff": np.ascontiguousarray(
            affinity[b].astype(np.float16)
            .reshape(K2, P, NS, W).transpose(1, 0, 2, 3)),
        "feat": np.ascontiguousarray(feature[b].reshape(H, W),
                                     dtype=np.float16),
        "smat": sm,
    } for b in range(B)]


def _run(affinity, feature, times, trace=False):
    from concourse.bass_utils import run_bass_kernel_spmd

    nc = _get_program(int(times))
    res = run_bass_kernel_spmd(nc, _in_maps(affinity, feature),
                               core_ids=list(range(B)), trace=trace)
    outs = np.stack([np.asarray(res.results[b]["out"]) for b in range(B)])
    return outs.reshape(B, 1, H, W).astype(np.float32), res


def kernel(affinity, feature, times) -> np.ndarray:
    affinity = np.asarray(affinity)
    feature = np.asarray(feature)
    assert affinity.shape == (B, K2, H, W), affinity.shape
    assert feature.shape[0] == B and feature.shape[-2:] == (H, W)
    out, _ = _run(affinity, feature, int(times))
    return out
